# revision 4
# baseline (speedup 1.0000x reference)
# Trainium2 Bass kernel for nn_LFGA_9363028706078 (dense_transformer), v3.
#
# Per batch b (one NeuronCore each, N = 4096 tokens):
#   q = Wq@fb + bq [16,N]; k = Wk@fb + bk [16,N]; v = Wv@fa + bv [64,N]
#   S[n,m] = q.k; attn = softmax_m(S); out = relu(gamma*(v@attn^T) + fa)
#
# v3 vs v2 (157.8us): the pipeline is ACT+DVE-exp-throughput-bound
# (~18.6us/chunk combined), so v3 gets the PE far under that roof and
# strips everything else off ACT/DVE:
#   - S^T via 4-way row-tiled K=32 matmuls (tile_position): 4 m-tiles run
#     concurrently, 259ns/4 vs 864ns full-mode (measured). Mode switches
#     tiled<->full cost ~173ns, HAM stays 8/8 (measured), so AV bursts are
#     batched 3 pairs at a time.
#   - PSUM: ring of 3 x [128,2,512] S tiles (6 banks) + acc (1) + proj (1).
#   - exp pair-granular: 9 ACT pairs (table exp -> fp8, DoubleRow AV) +
#     7 DVE pairs (Schraudolph bf16-bit i16, bf16 AV). Pair instrs measured
#     optimal (1114/1224 ns).
#   - acc rows 64:128 = 64 duplicated softmax denominators (extra wv ones
#     columns, free on PE) -> epilogue is: ACT bit-hack reciprocal
#     (i32: C - bits(x), +-5%), DVE mult (num PSUM x rden SBUF), GPS
#     add-fa + relu, DMA out. k is staged fully in the prologue; q is
#     projected one chunk ahead (1 ACT copy/chunk).
import os
import sys

import numpy as np

for _p in ("/opt/trn_rl_repo",):
    if _p not in sys.path and os.path.isdir(_p):
        sys.path.append(_p)

import ml_dtypes  # noqa: E402

import concourse.bass as bass  # noqa: E402
import concourse.tile as tile  # noqa: E402
from concourse import bacc, mybir  # noqa: E402

B, C, H, W = 8, 64, 64, 64
N = H * W  # 4096
CQK = 16
CA = C + 1  # 65 augmented channels (bias row)
P = 128
NC = 512  # chunk width
NCH = N // NC  # 8 chunks
NPAIR = 16  # m-tile pairs per chunk

DVE_PAIR_IDS = [1, 3, 5, 7, 9, 11, 13]
ACT_PAIR_IDS = [p for p in range(NPAIR) if p not in DVE_PAIR_IDS]
VT8_G = {p: i for i, p in enumerate(ACT_PAIR_IDS)}
VT16_G = {p: 2 * i for i, p in enumerate(DVE_PAIR_IDS)}

SCALE_V = 8.0
EXP_BIAS = float(np.log(1.0 / 32.0))  # ACT computes exp(S)/32 -> fp8
A16 = 128.0 / np.log(2.0)  # Schraudolph: i16(S*A+B) = bf16 bits of exp(S)/32
B16 = 127.0 * 128.0 - 5.0 * 128.0 - 5.25
C_RECIP = 0x7EF311C0  # bits(1/x) ~= C - bits(x), max rel err 5.1%
C_RECIP_F = float(np.float32(C_RECIP))

F32 = mybir.dt.float32
BF16 = mybir.dt.bfloat16
FP8 = mybir.dt.float8e4
I16 = mybir.dt.int16
I32 = mybir.dt.int32

AV_LAG = 2  # AV(p) emitted at burst >= p + AV_LAG
BURST_AT = (2, 5, 8, 11, 14)


def _build_program():
    nc = bacc.Bacc("TRN2", target_bir_lowering=False, debug=False, num_devices=B)

    fa32_d = nc.dram_tensor("fa32", [C, N], F32, kind="ExternalInput")
    fa16_d = nc.dram_tensor("fa16", [CA, N], BF16, kind="ExternalInput")
    fb16_d = nc.dram_tensor("fb16", [CA, N], BF16, kind="ExternalInput")
    wqk_d = nc.dram_tensor("wqk", [CA, 2, P], BF16, kind="ExternalInput")
    wv_d = nc.dram_tensor("wv", [CA, P], BF16, kind="ExternalInput")
    out_d = nc.dram_tensor("out", [C, N], F32, kind="ExternalOutput")

    with tile.TileContext(nc) as tc:
        with (
            tc.tile_pool(name="consts", bufs=1) as consts,
            tc.tile_pool(name="pg8", bufs=3) as pg8_pool,
            tc.tile_pool(name="pgb", bufs=3) as pgb_pool,
            tc.tile_pool(name="ep", bufs=2) as ep_pool,
        ):
            # ---------------- SBUF tensors ----------------
            fb16 = consts.tile([CA, N], BF16)
            fa16 = consts.tile([CA, N], BF16)
            fa32 = consts.tile([C, N], F32)
            wqk = consts.tile([CA, 2, P], BF16)
            wv = consts.tile([CA, P], BF16)
            qk16 = consts.tile([P, 2, N], BF16)  # [:,0]=q, [:,1]=k (x4 rep)
            vt8 = consts.tile([P, len(ACT_PAIR_IDS), 2, P], FP8)
            vt16 = consts.tile([P, 2 * len(DVE_PAIR_IDS), P], BF16)
            bias_t = consts.tile([P, 1], F32)
            zpad = consts.tile([P, P], BF16)
            wpad = consts.tile([P, NC], BF16)

            # ---------------- input DMA ----------------
            for c in range(NCH):
                eng = nc.sync if c % 2 == 0 else nc.gpsimd
                eng.dma_start(fb16[:, bass.ts(c, NC)], fb16_d[:, bass.ts(c, NC)])
            nc.gpsimd.dma_start(wqk[:], wqk_d[:])
            nc.gpsimd.dma_start(wv[:], wv_d[:])
            nc.sync.dma_start(fa16[:, 0 : N // 2], fa16_d[:, 0 : N // 2])
            nc.sync.dma_start(fa16[:, N // 2 : N], fa16_d[:, N // 2 : N])
            nc.sync.dma_start(fa32[:, 0 : N // 2], fa32_d[:, 0 : N // 2])
            nc.sync.dma_start(fa32[:, N // 2 : N], fa32_d[:, N // 2 : N])
            nc.vector.memset(bias_t[:], EXP_BIAS)
            nc.vector.memset(zpad[:], 0.0)
            nc.vector.memset(wpad[:], 0.0)

            # ---------------- prologue ----------------
            # PE warmup (full mode, zeros) while the first fb16 pieces land.
            with tc.tile_pool(name="wup", bufs=1, space="PSUM") as wup:
                wu = wup.tile([P, NC], F32)
                for i in range(12):
                    nc.tensor.matmul(
                        wu[:],
                        lhsT=zpad[:],
                        rhs=wpad[:],
                        start=(i == 0),
                        stop=(i == 11),
                        skip_group_check=True,
                    )

            psum_pools = tc.tile_pool(name="sps", bufs=3, space="PSUM")
            sps_pool = psum_pools.__enter__()
            acc_ctx = tc.tile_pool(name="accp", bufs=2, space="PSUM")
            acc_pool = acc_ctx.__enter__()

            # k for ALL chunks staged to SBUF up front (S of chunk 0 needs
            # every k m-tile). 2 k-chunks per ring tile; ACT/DVE copy halves.
            state_kst0 = []
            for j in range(4):
                kt = sps_pool.tile([P, 2, NC], F32, tag="sps", name=f"kst{j}")
                if j == 0:
                    state_kst0.append(kt)
                for h in range(2):
                    c = 2 * j + h
                    nc.tensor.matmul(
                        kt[:, h, :],
                        lhsT=wqk[:, 1, :],
                        rhs=fb16[:, bass.ts(c, NC)],
                        start=True,
                        stop=True,
                    )
                nc.scalar.activation(
                    out=qk16[:, 1, bass.ts(2 * j, NC)],
                    in_=kt[:, 0, :],
                    func=mybir.ActivationFunctionType.Copy,
                )
                nc.vector.tensor_copy(
                    out=qk16[:, 1, bass.ts(2 * j + 1, NC)], in_=kt[:, 1, :]
                )

            # q for chunk 0 (ring slot, bank 0 of the tile)
            q0 = sps_pool.tile([P, 2, NC], F32, tag="sps", name="q0")
            nc.tensor.matmul(
                q0[:, 0, :],
                lhsT=wqk[:, 0, :],
                rhs=fb16[:, 0:NC],
                start=True,
                stop=True,
            )
            nc.vector.tensor_copy(out=qk16[:, 0, 0:NC], in_=q0[:, 0, :])
            # v~ projection for all 32 m-tiles + fp8/bf16 casts (prologue:
            # overlaps k-staging copies; keeps every engine busy pre-chunk0)
            for h in range(8):
                vg = sps_pool.tile([P, 2, NC], F32, tag="sps", name=f"v{h}")
                for r in range(4):
                    mt = 4 * h + r
                    osl = vg[:, r // 2, P * (r % 2) : P * (r % 2) + P]
                    nc.tensor.matmul(
                        osl,
                        lhsT=fa16[:, bass.ts(mt, P)],
                        rhs=wv[:],
                        start=True,
                        stop=True,
                    )
                for half in range(2):
                    pr = 2 * h + half
                    sl = vg[:, half, 0 : 2 * P]
                    if pr in VT8_G:
                        nc.scalar.activation(
                            out=vt8[:, VT8_G[pr], :, :],
                            in_=sl,
                            func=mybir.ActivationFunctionType.Copy,
                        )
                    else:
                        g = VT16_G[pr]
                        nc.vector.tensor_copy(
                            out=vt16[:, g : g + 2, :], in_=sl
                        )
                nc.tensor.matmul(
                    vg[:, 0, :],
                    lhsT=zpad[:],
                    rhs=wpad[:],
                    start=True,
                    stop=True,
                    skip_group_check=True,
                )

            # keep-warm dummies while prologue copies drain (results unused)
            kst0 = state_kst0[0]
            for i in range(4):
                nc.tensor.matmul(
                    kst0[:, 0, :],
                    lhsT=zpad[:],
                    rhs=wpad[:],
                    start=True,
                    stop=True,
                    skip_group_check=True,
                )

            for i in range(3):
                nc.tensor.matmul(
                    kst0[:, 0, :],
                    lhsT=zpad[:],
                    rhs=wpad[:],
                    start=True,
                    stop=True,
                    skip_group_check=True,
                )

            # ---------------- main loop ----------------
            n_av_total = len(ACT_PAIR_IDS) + 2 * len(DVE_PAIR_IDS)  # 23
            state = {}  # ci -> dict(sps tiles, pg tiles, acc)

            def emit_S(ci, p, st):
                sp = sps_pool.tile([P, 2, NC], F32, tag="sps", name=f"s{ci}_{p}")
                st["sps"][p] = sp
                for r in range(2):
                    m = 2 * p + r
                    rg = 32 * (m % 4)
                    nc.tensor.matmul(
                        sp[:, r, :],
                        lhsT=qk16[rg : rg + 32, 1, bass.ts(m, P)],
                        rhs=qk16[rg : rg + 32, 0, bass.ts(ci, NC)],
                        start=True,
                        stop=True,
                        tile_position=(rg, 0),
                    )

            def emit_exp(ci, p, st):
                sp = st["sps"][p]
                if p in VT8_G:
                    pg = pg8_pool.tile([P, 2, NC], FP8, tag="pg8")
                    nc.scalar.activation(
                        out=pg[:],
                        in_=sp[:],
                        func=mybir.ActivationFunctionType.Exp,
                        scale=1.0,
                        bias=bias_t[:],
                    )
                else:
                    pg = pgb_pool.tile([P, 2, NC], I16, tag="pgb")
                    nc.vector.tensor_scalar(
                        out=pg[:],
                        in0=sp[:],
                        scalar1=A16,
                        scalar2=B16,
                        op0=mybir.AluOpType.mult,
                        op1=mybir.AluOpType.add,
                    )
                st["pg"][p] = pg

            def emit_AV(ci, p, st):
                acc = st["acc"]
                pg = st["pg"][p]
                first = st["av_idx"] == 0
                if p in VT8_G:
                    last = st["av_idx"] == n_av_total - 1
                    nc.tensor.matmul(
                        acc[:],
                        lhsT=vt8[:, VT8_G[p], :, :],
                        rhs=pg[:],
                        start=first,
                        stop=last,
                        perf_mode=mybir.MatmulPerfMode.DoubleRow,
                    )
                    st["av_idx"] += 1
                else:
                    g = VT16_G[p]
                    for r in range(2):
                        last = st["av_idx"] == n_av_total - 1
                        nc.tensor.matmul(
                            acc[:],
                            lhsT=vt16[:, g + r, :],
                            rhs=pg[:, r, :].bitcast(BF16),
                            start=first and r == 0,
                            stop=last,
                        )
                        st["av_idx"] += 1

            def emit_epilogue(ci, st):
                # DVE bit-hack reciprocal of the 64 dup denominator rows
                acc = st["acc"]
                rden = ep_pool.tile([C, NC], I32, tag="rden", name=f"rd{ci}")
                nc.vector.tensor_scalar(
                    out=rden[:],
                    in0=acc[C:P, :].bitcast(I32),
                    scalar1=-1,
                    scalar2=C_RECIP,
                    op0=mybir.AluOpType.mult,
                    op1=mybir.AluOpType.add,
                )
                tnum = ep_pool.tile([C, NC], F32, tag="tnum", name=f"tn{ci}")
                nc.vector.tensor_tensor(
                    out=tnum[:],
                    in0=acc[0:C, :],
                    in1=rden[:].bitcast(F32),
                    op=mybir.AluOpType.mult,
                )
                uout = ep_pool.tile([C, NC], F32, tag="uout", name=f"uo{ci}")
                nsl = bass.ts(ci, NC)
                nc.gpsimd.tensor_add(out=uout[:], in0=tnum[:], in1=fa32[:, nsl])
                nc.scalar.activation(
                    out=uout[:],
                    in_=uout[:],
                    func=mybir.ActivationFunctionType.Relu,
                )
                nc.sync.dma_start(out=out_d[:, nsl], in_=uout[:])

            pend = []  # global AV queue of (ci, p)
            for ci in range(NCH):
                st = {"sps": {}, "pg": {}, "av_idx": 0,
                      "acc": acc_pool.tile([P, NC], F32, tag="acc",
                                           name=f"acc{ci}")}
                state[ci] = st
                prev = state.get(ci - 1)
                av_lag = AV_LAG

                for p in range(NPAIR):
                    emit_S(ci, p, st)
                    emit_exp(ci, p, st)
                    pend.append((ci, p))
                    if p == 3 and prev is not None:
                        emit_epilogue(ci - 1, prev)
                    if p in BURST_AT:
                        # full-mode burst
                        while pend and (pend[0][0] < ci
                                        or pend[0][1] <= p - av_lag):
                            qci, qp = pend.pop(0)
                            emit_AV(qci, qp, state[qci])
                        if p == 8 and ci < NCH - 1:
                            # q projection for next chunk (ring slot)
                            qn = sps_pool.tile(
                                [P, 2, NC], F32, tag="sps", name=f"q{ci + 1}"
                            )
                            nc.tensor.matmul(
                                qn[:, 0, :],
                                lhsT=wqk[:, 0, :],
                                rhs=fb16[:, bass.ts(ci + 1, NC)],
                                start=True,
                                stop=True,
                            )
                            nc.vector.tensor_copy(
                                out=qk16[:, 0, bass.ts(ci + 1, NC)],
                                in_=qn[:, 0, :],
                            )
            # drain remaining AVs + last epilogue
            while pend:
                qci, qp = pend.pop(0)
                emit_AV(qci, qp, state[qci])
            emit_epilogue(NCH - 1, state[NCH - 1])

            acc_ctx.__exit__(None, None, None)
            psum_pools.__exit__(None, None, None)

    nc.compile()
    return nc


_NC = None


def _get_program():
    global _NC
    if _NC is None:
        _NC = _build_program()
    return _NC


def _host_prep(fa, fb, Wq, bq, Wk, bk, Wv, bv, gamma):
    fa = np.asarray(fa, dtype=np.float32)
    fb = np.asarray(fb, dtype=np.float32)
    Wq = np.asarray(Wq, dtype=np.float64)
    bq = np.asarray(bq, dtype=np.float64)
    Wk = np.asarray(Wk, dtype=np.float64)
    bk = np.asarray(bk, dtype=np.float64)
    Wv = np.asarray(Wv, dtype=np.float64)
    bv = np.asarray(bv, dtype=np.float64)
    gamma = float(np.asarray(gamma).reshape(-1)[0])

    Wq_aug = np.concatenate([Wq, bq[:, None]], axis=1)  # [16, 65]
    Wk_aug = np.concatenate([Wk, bk[:, None]], axis=1)
    wqk = np.zeros((CA, 2, P), dtype=np.float64)
    for g in range(4):
        wqk[:, 0, 32 * g : 32 * g + CQK] = Wq_aug.T
        wqk[:, 1, 32 * g : 32 * g + CQK] = Wk_aug.T
    wqk16 = wqk.astype(ml_dtypes.bfloat16)

    wva = np.zeros((CA, P), dtype=np.float64)
    wva[0:C, 0:C] = SCALE_V * gamma * Wv.T
    wva[C, 0:C] = SCALE_V * gamma * bv
    wva[C, C:P] = SCALE_V  # 64 duplicated denominator columns
    wv16 = wva.astype(ml_dtypes.bfloat16)

    ones_row = np.ones((1, N), dtype=np.float32)
    in_maps = []
    for b in range(B):
        fa_flat = fa[b].reshape(C, N)
        fb_flat = fb[b].reshape(C, N)
        fa_aug = np.concatenate([fa_flat, ones_row], axis=0)
        fb_aug = np.concatenate([fb_flat, ones_row], axis=0)
        in_maps.append(
            {
                "fa32": np.ascontiguousarray(fa_flat),
                "fa16": fa_aug.astype(ml_dtypes.bfloat16),
                "fb16": fb_aug.astype(ml_dtypes.bfloat16),
                "wqk": wqk16,
                "wv": wv16,
            }
        )
    return in_maps


def _run(inputs, trace=False):
    from concourse.bass_utils import run_bass_kernel_spmd

    nc = _get_program()
    in_maps = _host_prep(**inputs)
    res = run_bass_kernel_spmd(nc, in_maps, core_ids=list(range(B)), trace=trace)
    out = np.stack([res.results[b]["out"].reshape(C, H, W) for b in range(B)])
    return out, res


def kernel(**inputs) -> np.ndarray:
    out, _ = _run(inputs, trace=False)
    return out


def kernel_traced(**inputs):
    return _run(inputs, trace=True)
